# revision 64
# baseline (speedup 1.0000x reference)
"""Trainium2 Bass kernel for a MixEncoderLayer (attention w/ additive cost
matrix bias + FFN), batch 8, seq 1024, d_model 512, 8 heads, d_ff 2048.

Strategy: pure data parallelism — one batch element per NeuronCore, no
collectives.  Main design points:

  * All matmul operands are FP16 — the PE streams 16-bit moving operands at
    2 cols/cycle vs 1 for f32r, halving matmul time.  PSUM stays fp32.

  * All weight/cost/input transposes and fp16 casts happen on the HOST in
    kernel() (weights are shared across cores; per-core transposes are cheap
    numpy).  The 1/sqrt(dk) scale is pre-folded into wq.  This removes all
    PE load-transposes and their PSUM->SBUF evictions from the device.

  * Key-major attention: scores^T[k,q]; cost^T preloaded into PSUM via fp16
    identity matmul, QK^T accumulates on top, ACT exps a 2-bank PSUM tile
    into fp16 sc, which is directly the moving operand of the attn@V
    accumulation.  Rowsums ride shotgun via augmented [V|1] stationary.

  * Residual adds and the b2 bias are folded into PSUM with identity /
    rank-1 matmuls (PE is cheaper than another DVE pass); layernorm runs
    directly on the PSUM accumulator: bn_stats + one dual-scalar
    tensor_scalar normalize, gain/bias applied as fp16 ops on Pool.

  * Software pipelining: the QKV projections are interleaved into the
    c=0 attention half, and stage D (fc+LN1) + stage E (FFN) of the c=0
    token half are interleaved into the c=1 attention half, so the PE works
    through the ACT-exp-paced attention phases.  attn_out transposes use
    the XBAR DMA-transpose path (no PE/PSUM).
"""

import numpy as np

import concourse.bass as bass
import concourse.mybir as mybir
import concourse.tile as tile
from concourse.masks import make_identity

F32 = mybir.dt.float32
F16 = mybir.dt.float16
AF = mybir.ActivationFunctionType
ALU = mybir.AluOpType

S, Dm, H, DK, DF = 1024, 512, 8, 64, 2048
ST, DT, FT = S // 128, Dm // 128, DF // 128  # 8, 4, 16
NCORES = 8
LN_EPS = 1e-6

INPUT_SPECS = {
    "xt16": ((Dm, S), F16),       # enc_input^T
    "x16": ((S, Dm), F16),        # enc_input (residual)
    "costt16": ((S, S), F16),     # cost_mat^T
    "wqt16": ((Dm, Dm), F16),     # (wq/8)^T
    "wkt16": ((Dm, Dm), F16),     # wk^T
    "wvt16": ((Dm, Dm), F16),     # wv^T
    "fcwt16": ((Dm, Dm), F16),    # fc_w^T
    "w1t16": ((Dm, DF), F16),     # w1^T
    "w2t16": ((DF, Dm), F16),     # w2^T
    "b1r": ((128, FT), F32),      # b1 reshaped: [p, jt] = b1[jt*128+p]
    "lnrows16": ((5, Dm), F16),   # [ln1_g, ln1_b, ln2_g, ln2_b, b2]
}


def _build(tc, io, out_ap):
    nc = tc.nc
    with nc.allow_low_precision(reason="fp16 matmul operands; accumulation stays f32 in PSUM"):
        _build_inner(tc, io, out_ap)


def _build_inner(tc, io, out_ap):
    nc = tc.nc

    # ---------------- long-lived pools ----------------
    singles = tc.alloc_tile_pool(name="singles", bufs=1, side="left")
    p_in = tc.alloc_tile_pool(name="p_in", bufs=1, side="left")
    p_qkv = tc.alloc_tile_pool(name="p_qkv", bufs=1, side="left")
    p_mid = tc.alloc_tile_pool(name="p_mid", bufs=1, side="left")
    p_work = tc.alloc_tile_pool(name="p_work", bufs=2, side="right")
    # PSUM: 4 + 2 + 2 = 8 banks
    scpsW = tc.alloc_tile_pool(name="scpsW", bufs=2, space="PSUM", side="right")
    cpsp = tc.alloc_tile_pool(name="cpsp", bufs=2, space="PSUM", side="right")
    dps = tc.alloc_tile_pool(name="dps", bufs=2, space="PSUM", side="right")

    ident16 = singles.tile([128, 128], F16, tag="ident16")
    make_identity(nc, ident16)
    eps_t = singles.tile([128, 1], F32, tag="eps")
    nc.gpsimd.memset(eps_t, LN_EPS)
    ones16 = singles.tile([128, 64], F16, tag="ones16")
    nc.gpsimd.memset(ones16, 1.0)
    ones_row16 = singles.tile([1, 128], F16, tag="ones_row16")
    nc.gpsimd.memset(ones_row16, 1.0)

    # ---------------- loads (host-prepped layouts, no device transposes) ---
    def sbload(queue, dram, shape, n, tag, dtype=F16):
        ts = []
        for i in range(n):
            t = p_in.tile(list(shape), dtype, tag=f"{tag}{i}", name=f"{tag}{i}")
            queue.dma_start(out=t, in_=dram[i * 128:(i + 1) * 128, :])
            ts.append(t)
        return ts

    # Early loads interleaved across BOTH hwdge queues so the eager QKV
    # matmuls can start ~3us in; late loads go on SP (ACT queue must be
    # free before the first exp).
    def sbload2(queues, dram, shape, n, tag, dtype=F16):
        ts = []
        for i in range(n):
            t = p_in.tile(list(shape), dtype, tag=f"{tag}{i}", name=f"{tag}{i}")
            queues[i % len(queues)].dma_start(
                out=t, in_=dram[i * 128:(i + 1) * 128, :])
            ts.append(t)
        return ts

    def ld(queue, dram, r0, c0_, shape, tag):
        t = p_in.tile(list(shape), F16, tag=tag, name=tag)
        queue.dma_start(out=t, in_=dram[r0:r0 + shape[0], c0_:c0_ + shape[1]])
        return t

    # enc^T is loaded as per-c-half tiles so the first QK matmuls start ~4us
    # in; load order is hand-interleaved across both hwdge queues by need.
    xtc = [[None] * DT for _ in range(2)]
    wkt, wqt, wvt = [None] * DT, [None] * DT, [None] * DT
    costT = [None] * ST
    # NOTHING loads on the ACT queue: a dma_start occupies the issuing SEQ
    # for ~1.3us (DGE config + descriptor gen), and any load traced before
    # the exps would delay the whole attention pipeline.  SP (HWDGE) and
    # gpsimd (SWDGE) are the two load channels.
    # SP queue: eager-K/Q inputs, then cost/V, then stage-D/E weights.
    xtc[0][0] = ld(nc.sync, io["xt16"], 0, 0, (128, 512), "xt00")
    wkt[0] = ld(nc.sync, io["wkt16"], 0, 0, (128, Dm), "wkt0")
    wqt[0] = ld(nc.sync, io["wqt16"], 0, 0, (128, Dm), "wqt0")
    xtc[0][2] = ld(nc.sync, io["xt16"], 256, 0, (128, 512), "xt02")
    wkt[2] = ld(nc.sync, io["wkt16"], 256, 0, (128, Dm), "wkt2")
    wqt[2] = ld(nc.sync, io["wqt16"], 256, 0, (128, Dm), "wqt2")
    wvt_m = p_in.tile([128, DT, Dm], F16, tag="wvtm", name="wvtm")
    nc.sync.dma_start(out=wvt_m, in_=io["wvt16"].rearrange(
        "(j p) o -> p j o", p=128))
    wvt = [wvt_m[:, d, :] for d in range(DT)]
    ctm1 = p_in.tile([128, 2, S], F16, tag="ctm1", name="ctm1")
    nc.sync.dma_start(out=ctm1, in_=io["costt16"][256:512, :].rearrange(
        "(j p) o -> p j o", p=128))
    costT[2], costT[3] = ctm1[:, 0, :], ctm1[:, 1, :]
    xtc1_m = p_in.tile([128, DT, 512], F16, tag="xtc1m", name="xtc1m")
    nc.sync.dma_start(out=xtc1_m, in_=io["xt16"][:, 512:].rearrange(
        "(j p) o -> p j o", p=128))
    for _d in range(DT):
        xtc[1][_d] = xtc1_m[:, _d, :]
    ctm2 = p_in.tile([128, 2, S], F16, tag="ctm2", name="ctm2")
    nc.sync.dma_start(out=ctm2, in_=io["costt16"][512:768, :].rearrange(
        "(j p) o -> p j o", p=128))
    costT[4], costT[5] = ctm2[:, 0, :], ctm2[:, 1, :]
    ctm3 = p_in.tile([128, 2, S], F16, tag="ctm3", name="ctm3")
    nc.sync.dma_start(out=ctm3, in_=io["costt16"][768:1024, :].rearrange(
        "(j p) o -> p j o", p=128))
    costT[6], costT[7] = ctm3[:, 0, :], ctm3[:, 1, :]
    # gpsimd (SWDGE) queue
    xtc[0][1] = ld(nc.gpsimd, io["xt16"], 128, 0, (128, 512), "xt01")
    wkt[1] = ld(nc.gpsimd, io["wkt16"], 128, 0, (128, Dm), "wkt1")
    xtc[0][3] = ld(nc.gpsimd, io["xt16"], 384, 0, (128, 512), "xt03")
    wkt[3] = ld(nc.gpsimd, io["wkt16"], 384, 0, (128, Dm), "wkt3")
    costT[0] = ld(nc.gpsimd, io["costt16"], 0, 0, (128, S), "ct0")
    wqt[1] = ld(nc.gpsimd, io["wqt16"], 128, 0, (128, Dm), "wqt1")
    wqt[3] = ld(nc.gpsimd, io["wqt16"], 384, 0, (128, Dm), "wqt3")
    costT[1] = ld(nc.gpsimd, io["costt16"], 128, 0, (128, S), "ct1")
    fcw_m = p_in.tile([128, DT, Dm], F16, tag="fcwm", name="fcwm")
    nc.gpsimd.dma_start(out=fcw_m, in_=io["fcwt16"].rearrange(
        "(j p) o -> p j o", p=128))
    fcwt = [fcw_m[:, d, :] for d in range(DT)]
    x16_m = []
    for _g in range(2):
        _t = p_in.tile([128, 4, Dm], F16, tag=f"x16m{_g}", name=f"x16m{_g}")
        nc.gpsimd.dma_start(
            out=_t, in_=io["x16"][_g * 512:(_g + 1) * 512, :].rearrange(
                "(j p) o -> p j o", p=128))
        x16_m.append(_t)
    x16 = [x16_m[st // 4][:, st % 4, :] for st in range(ST)]
    w1t = sbload2([nc.sync], io["w1t16"], (128, DF), DT, "w1t")
    w2t_m = []
    for _g in range(4):
        _t = p_in.tile([128, 4, Dm], F16, tag=f"w2tm{_g}", name=f"w2tm{_g}")
        nc.sync.dma_start(
            out=_t, in_=io["w2t16"][_g * 512:(_g + 1) * 512, :].rearrange(
                "(j p) o -> p j o", p=128))
        w2t_m.append(_t)
    w2t = [w2t_m[jt // 4][:, jt % 4, :] for jt in range(FT)]

    lnrows = singles.tile([128, 5, Dm], F16, tag="lnrows", name="lnrows")
    nc.gpsimd.dma_start(
        out=lnrows, in_=io["lnrows16"][None, :, :].to_broadcast((128, 5, Dm)))
    ln1g_r, ln1b_r = lnrows[:, 0, :], lnrows[:, 1, :]
    ln2g_r, ln2b_r = lnrows[:, 2, :], lnrows[:, 3, :]
    b2row = lnrows[0:1, 4, :]
    b1r = singles.tile([128, FT], F32, tag="b1r", name="b1r")
    nc.gpsimd.dma_start(out=b1r, in_=io["b1r"])

    # ---------------- long-lived intermediates ----------------
    QT = [p_qkv.tile([128, S], F16, tag=f"qt{i}", name=f"qt{i}") for i in range(DT)]
    KT = [p_qkv.tile([128, S], F16, tag=f"kt{i}", name=f"kt{i}") for i in range(DT)]
    vaug = [p_qkv.tile([128, H, DK + 1], F16, tag=f"va{st}", name=f"va{st}")
            for st in range(ST)]
    ctxT = [p_mid.tile([128, S], F16, tag=f"cx{i}", name=f"cx{i}") for i in range(DT)]
    attn_out = [p_mid.tile([128, Dm], F16, tag=f"ao{st}", name=f"ao{st}")
                for st in range(ST)]
    # aoT_all[p, d, s] = attn_out[s // 128][s % 128... ] transposed: filled by
    # XBAR DMA transposes, one per token tile: out[p, d, q] = in[q, d*128+p]
    aoT_all = p_mid.tile([128, DT, S], F16, tag="aot", name="aot")
    h1T = [p_mid.tile([128, 512], F16, tag=f"h1t{jt}", name=f"h1t{jt}")
           for jt in range(FT)]

    # ---------------- step closures ----------------
    def q_step(it, c):
        ps = dps.tile([128, 512], F32, tag="dps", name="q_ps")
        for d in range(DT):
            nc.tensor.matmul(ps, wqt[d][:, it * 128:(it + 1) * 128],
                             xtc[c][d],
                             start=(d == 0), stop=(d == DT - 1))
        nc.scalar.copy(QT[it][:, c * 512:(c + 1) * 512], ps)

    def k_step(it, c):
        ps = dps.tile([128, 512], F32, tag="dps", name="k_ps")
        for d in range(DT):
            nc.tensor.matmul(ps, wkt[d][:, it * 128:(it + 1) * 128],
                             xtc[c][d],
                             start=(d == 0), stop=(d == DT - 1))
        nc.vector.tensor_copy(KT[it][:, c * 512:(c + 1) * 512], ps)

    def v_step(st):
        nc.vector.memset(
            vaug[st][:, :, DK:DK + 1].rearrange("p h o -> p (h o)"), 1.0)
        ps = dps.tile([128, 512], F32, tag="dps", name="v_ps")
        sc_, so = st // 4, (st % 4) * 128
        for d in range(DT):
            nc.tensor.matmul(ps, xtc[sc_][d][:, so:so + 128], wvt[d],
                             start=(d == 0), stop=(d == DT - 1))
        nc.vector.tensor_copy(
            out=vaug[st][:, :, 0:DK],
            in_=ps.rearrange("p (h e) -> p h e", h=H))

    # -------- batched LayerNorm: stats per tile, one Newton rsqrt per 4 ----
    # ACT is kept exp-only (plus relu, same table set) — a Sqrt would force
    # an activation-table reload (1.3us) on every exp<->sqrt alternation in
    # the interleaved schedule.  istd = rsqrt(var+eps) is computed on DVE:
    # y0 = 1/(0.5(v+eps)+0.5), then 3 Newton steps (rel err <2e-6 for
    # v in [0.5,3]).
    def ln_stats(ps, mvb, slot, xsub_on_act=False):
        """bn stats of PSUM tile -> mvb[:, :, slot]; returns xsub tile
        (ps - mean, f16).  xsub_on_act routes the fat subtract pass to ACT
        (identity+bias) for tail phases where ACT is idle and DVE is the
        bottleneck."""
        stats = p_work.tile([128, 6], F32, tag="ln_stats", bufs=6, name="ln_stats")
        nc.vector.bn_stats(out=stats, in_=ps)
        nc.vector.bn_aggr(out=mvb[:, :, slot], in_=stats)
        xsub = p_work.tile([128, Dm], F16, tag="ln_xsub", bufs=8, name="ln_xsub")
        if xsub_on_act:
            negm = p_work.tile([128, 1], F32, tag="ln_negm", bufs=6,
                               name="ln_negm")
            nc.vector.tensor_scalar(out=negm, in0=mvb[:, 0:1, slot],
                                    scalar1=-1.0, scalar2=None, op0=ALU.mult)
            nc.scalar.activation(out=xsub, in_=ps, func=AF.Identity, bias=negm)
        else:
            nc.vector.tensor_scalar(out=xsub, in0=ps,
                                    scalar1=mvb[:, 0:1, slot], scalar2=None,
                                    op0=ALU.subtract)
        return xsub

    def ln_newton(mvb, n):
        """istd[128, n] = rsqrt(var + eps) via DVE-only Newton."""
        v = mvb[:, 1:2, :].rearrange("p o n -> p (o n)")
        vp = p_work.tile([128, n], F32, tag="ln_vp", bufs=4, name="ln_vp")
        nc.vector.tensor_scalar(out=vp, in0=v, scalar1=float(LN_EPS),
                                scalar2=None, op0=ALU.add)
        y = p_work.tile([128, n], F32, tag="ln_y0", bufs=4, name="ln_y0")
        nc.vector.tensor_scalar(out=y, in0=vp, scalar1=0.5, scalar2=0.5,
                                op0=ALU.mult, op1=ALU.add)
        nc.vector.reciprocal(out=y, in_=y)
        for _ in range(2):
            a = p_work.tile([128, n], F32, tag="ln_a", bufs=4, name="ln_a")
            nc.vector.tensor_tensor(out=a, in0=y, in1=y, op=ALU.mult)
            nc.vector.tensor_tensor(out=a, in0=a, in1=vp, op=ALU.mult)
            nc.vector.tensor_scalar(out=a, in0=a, scalar1=-0.5, scalar2=1.5,
                                    op0=ALU.mult, op1=ALU.add)
            nc.vector.tensor_tensor(out=y, in0=y, in1=a, op=ALU.mult)
        return y

    def ln_apply(xsub, istd_col, dst, g16, b16, final_f32, use_pool=False,
                 y_dst=None):
        xn = p_work.tile([128, Dm], F16, tag="ln_xn", bufs=4, name="ln_xn")
        nc.vector.tensor_scalar(out=xn, in0=xsub, scalar1=istd_col,
                                scalar2=None, op0=ALU.mult)
        xg = p_work.tile([128, Dm], F16, tag="ln_xg", bufs=4, name="ln_xg")
        if final_f32:
            # keep the output chain off Pool (tail latency)
            nc.vector.tensor_tensor(out=xg, in0=xn, in1=g16, op=ALU.mult)
            nc.vector.tensor_tensor(out=y_dst, in0=xg, in1=b16, op=ALU.add)
            return y_dst
        eng = nc.gpsimd if use_pool else nc.vector
        eng.tensor_tensor(out=xg, in0=xn, in1=g16, op=ALU.mult)
        eng.tensor_tensor(out=dst, in0=xg, in1=b16, op=ALU.add)
        return dst

    def fc_mm_step(st, mvb, slot, xsubs):
        ps = dps.tile([128, 512], F32, tag="dps", name="fc_ps")
        nc.tensor.matmul(ps, ident16, x16[st], start=True, stop=False)
        for et in range(DT):
            nc.tensor.matmul(ps, ctxT[et][:, st * 128:(st + 1) * 128],
                             fcwt[et], start=False, stop=(et == DT - 1))
        xsubs[slot] = ln_stats(ps, mvb, slot)

    def ln1_finish_step(c, mvb, xsubs):
        istd = ln_newton(mvb, 4)
        for i in range(4):
            # all-DVE: Pool's serial ~1.1us/op chain stalls the in-order PE
            # queue via the ln1 -> aoT -> FFN1 fill dependency
            ln_apply(xsubs[i], istd[:, i:i + 1], attn_out[c * 4 + i],
                     ln1g_r, ln1b_r, final_f32=False, use_pool=False)

    def aot_step(c, both_queues=False):
        # XBAR DMA transposes; use the ACT queue too only when ACT is idle
        # (a scalar-queue DMA issue would delay queued exps otherwise)
        for i in range(4):
            st = c * 4 + i
            q = nc.scalar if (both_queues and i % 2) else nc.sync
            q.dma_start_transpose(
                out=aoT_all[:, :, st * 128:(st + 1) * 128],
                in_=attn_out[st])

    def f1_step(c2, jt):
        ps = dps.tile([128, 512], F32, tag="dps", name="f1_ps")
        for d in range(DT):
            nc.tensor.matmul(ps, w1t[d][:, jt * 128:(jt + 1) * 128],
                             aoT_all[:, d, c2 * 512:(c2 + 1) * 512],
                             start=(d == 0), stop=(d == DT - 1))
        # bias+relu eviction: split DVE/ACT mid-attention (Relu shares Exp's
        # table set); in the c2=1 tail ACT is idle and DVE is the pacer, so
        # ACT takes 3 in 4.
        # c2=1 runs in the tail where ACT is otherwise idle and DVE carries
        # the LN2 chains -> all-ACT there
        act_it = True if c2 == 1 else (jt % 2 == 0)
        if act_it:
            nc.scalar.activation(out=h1T[jt], in_=ps, func=AF.Relu,
                                 bias=b1r[:, jt:jt + 1], scale=1.0)
        else:
            nc.vector.tensor_scalar(out=h1T[jt], in0=ps,
                                    scalar1=b1r[:, jt:jt + 1],
                                    scalar2=0.0, op0=ALU.add, op1=ALU.max)

    def f2_mm_step(c2, sti, mvb, slot, xsubs, tail=False):
        st = c2 * 4 + sti
        ps = dps.tile([128, 512], F32, tag="dps", name="f2_ps")
        nc.tensor.matmul(ps, ident16, attn_out[st], start=True, stop=False)
        nc.tensor.matmul(ps, ones_row16, b2row, start=False, stop=False)
        for jt in range(FT):
            nc.tensor.matmul(ps, h1T[jt][:, sti * 128:(sti + 1) * 128],
                             w2t[jt], start=False, stop=(jt == FT - 1))
        xsubs[slot] = ln_stats(ps, mvb, slot, xsub_on_act=tail)

    def ln2_finish_step(c2, sti0, mvb, xsubs, n, tail=False):
        istd = ln_newton(mvb, n)
        # write the batch's outputs into one paired tile -> ONE out-DMA per
        # batch (each dma_start costs ~1.3us of SP.SEQ at the tail)
        y2 = p_work.tile([128, n, Dm], F32, tag="ln_y2", bufs=3, name="ln_y2")
        for i in range(n):
            ln_apply(xsubs[i], istd[:, i:i + 1], None,
                     ln2g_r, ln2b_r, final_f32=True, y_dst=y2[:, i, :])
        st0 = c2 * 4 + sti0
        nc.sync.dma_start(
            out=out_ap[st0 * 128:(st0 + n) * 128, :].rearrange(
                "(j p) o -> p j o", p=128),
            in_=y2)

    # ---------------- attention with fill interleaving ----------------
    # The attn@V matmuls are software-pipelined one tile behind the QK
    # matmuls: the PE queue is in-order, so emitting av(t) (which waits on
    # exp(t)) before preload(t+1) would serialize PE behind ACT.  Deferred
    # av lets the PE prepare psW(t+1) while exp(t) runs -> ACT back-to-back.
    def attention(c, fill, pops_per_t):
        def do_norm(cps, hp):
            for hi in range(2):
                rsb = p_work.tile([65, 512], F16, tag="rsb", bufs=6, name="rsb")
                nc.vector.reciprocal(out=rsb[64:65, :],
                                     in_=cps[hi][DK:DK + 1, :])
                bps2 = dps.tile([64, 512], F32, tag="dps", name="bcps")
                nc.tensor.matmul(bps2, ones16[64:65, :], rsb[64:65, :],
                                 start=True, stop=True)
                bc = p_work.tile([64, 512], F16, tag="bc", bufs=3, name="bc")
                nc.vector.tensor_copy(bc, bps2)
                nc.vector.tensor_tensor(
                    out=ctxT[hp][hi * 64:(hi + 1) * 64, c * 512:(c + 1) * 512],
                    in0=cps[hi][0:DK, :], in1=bc, op=ALU.mult)

        def do_av(prev):
            cps, hp, t, sc = prev
            for hi in range(2):
                nc.tensor.matmul(
                    cps[hi], vaug[t][:, 2 * hp + hi, :],
                    sc[:, hi * 512:(hi + 1) * 512],
                    start=(t == 0), stop=(t == ST - 1))
            if t == ST - 1:
                do_norm(cps, hp)

        prev = None
        for hp in range(H // 2):
            cps = [cpsp.tile([DK + 1, 512], F32, tag="cps", name="cps")
                   for _ in range(2)]
            for t in range(ST):
                for _ in range(pops_per_t):
                    if fill:
                        fill.pop(0)()
                psW = scpsW.tile([128, 1024], F32, tag="scpsW", name="scpsW")
                for hi in range(2):
                    sl = psW[:, hi * 512:(hi + 1) * 512]
                    nc.tensor.matmul(sl, ident16,
                                     costT[t][:, c * 512:(c + 1) * 512],
                                     start=True, stop=False)
                    nc.tensor.matmul(
                        sl,
                        KT[hp][hi * 64:(hi + 1) * 64, t * 128:(t + 1) * 128],
                        QT[hp][hi * 64:(hi + 1) * 64, c * 512:(c + 1) * 512],
                        start=False, stop=True)
                sc = p_work.tile([128, 1024], F16, tag="sc", bufs=5, name="sc")
                nc.scalar.activation(out=sc, in_=psW, func=AF.Exp)
                if prev is not None:
                    do_av(prev)
                prev = (cps, hp, t, sc)
        do_av(prev)

    # ---------------- schedule ----------------
    # eager QKV for attention(c=0, hp=0) key-tiles t=0..3; the rest rides
    # the fill queue (K columns 512+ are only touched from t=4).
    k_step(0, 0)
    q_step(0, 0)
    v_step(0)

    fill_c0 = [
        lambda: v_step(1), lambda: k_step(0, 1),
        lambda: v_step(2), lambda: v_step(3),
        lambda: k_step(1, 0), lambda: v_step(4),
        lambda: k_step(1, 1), lambda: v_step(5),
        lambda: q_step(1, 0), lambda: v_step(6),
        lambda: v_step(7), lambda: k_step(2, 0),
        lambda: k_step(2, 1), lambda: q_step(2, 0),
        lambda: k_step(3, 0), lambda: k_step(3, 1),
        lambda: q_step(3, 0), lambda: q_step(0, 1),
        lambda: q_step(1, 1), lambda: q_step(2, 1),
        lambda: q_step(3, 1),
    ]
    attention(0, fill_c0, pops_per_t=2)
    while fill_c0:
        fill_c0.pop(0)()

    # D/E of the c=0 half interleaves into attention(c=1); LN2 runs in 2x2
    # batches, and each half's last f2 batch is held back so it can fill the
    # other half's ln1->aoT->FFN1 dependency hole in the tail.
    def de_steps(c):
        mvb1 = p_work.tile([128, 2, 4], F32, tag="ln_mvb", bufs=8, name="ln_mvb")
        xs1 = [None] * 4
        pre = [lambda i=i: fc_mm_step(c * 4 + i, mvb1, i, xs1)
               for i in range(4)]
        pre.append(lambda: ln1_finish_step(c, mvb1, xs1))
        pre.append(lambda: aot_step(c, both_queues=False))
        mid = [lambda jt=jt: f1_step(c, jt) for jt in range(FT)]
        batches = []
        # the very last tokens run per-tile so the final tile's LN2 chain
        # starts immediately after its own stats (shorter drain)
        groups = [(0, 2), (2, 2)] if c == 0 else [(0, 2), (2, 1), (3, 1)]
        for sti0, n in groups:
            tail = (c == 1) or (sti0 == 2)  # runs in the tail region
            tail_x = tail
            mvb2 = p_work.tile([128, 2, n], F32, tag="ln_mvb",
                               bufs=8, name="ln_mvb")
            xs2 = [None] * n
            trio = [lambda sti=sti0 + k, m=mvb2, k=k, x=xs2, tl=tail_x:
                    f2_mm_step(c, sti, m, k, x, tail=tl) for k in range(n)]
            trio.append(lambda s0=sti0, m=mvb2, x=xs2, nn_=n, tl=tail:
                        ln2_finish_step(c, s0, m, x, nn_, tail=tl))
            batches.append(trio)
        flat_rest = [s for b in batches[1:] for s in b]
        return pre, mid + batches[0], flat_rest

    pre0, mid0, held0 = de_steps(0)
    # hold back the last 4 FFN1 steps of c=0 (and everything downstream of
    # them) as extra tail filler
    fill_c1 = pre0 + mid0[:12]
    held0 = mid0[12:] + held0
    attention(1, fill_c1, pops_per_t=1)
    while fill_c1:
        fill_c1.pop(0)()

    # tail: stage D/E for the second token half; c=0's held steps fill the
    # ln1(c=1) -> aoT -> FFN1 latency hole.
    pre1, mid1, held1 = de_steps(1)
    for s in pre1:
        s()
    for s in held0:
        s()
    for s in mid1:
        s()
    for s in held1:
        s()

    # release (LIFO per side)
    dps.release()
    cpsp.release()
    scpsW.release()
    p_work.release()
    p_mid.release()
    p_qkv.release()
    p_in.release()
    singles.release()


def build_nc(iters=1):
    """iters>1 unrolls the whole kernel body N times (separate DRAM outputs
    per iteration) — used only by the timing harness to difference away
    dispatch/transfer overhead."""
    from concourse import bacc

    nc = bacc.Bacc("TRN2", target_bir_lowering=False, debug=False)
    io = {
        name: nc.dram_tensor(name, list(shape), dt, kind="ExternalInput").ap()
        for name, (shape, dt) in INPUT_SPECS.items()
    }
    # all unrolled iterations write the SAME output tensor: iterations are
    # value-identical, and a single ExternalOutput keeps the PJRT buffer
    # bookkeeping constant so wall-clock differencing isolates device time
    out0 = nc.dram_tensor("out", [S, Dm], F32, kind="ExternalOutput").ap()
    out_aps = [out0] * iters
    with tile.TileContext(nc) as tc:
        for it in range(iters):
            _build(tc, io, out_aps[it])
    nc.compile()
    return nc


_NC_CACHE = None


def get_nc():
    global _NC_CACHE
    if _NC_CACHE is None:
        _NC_CACHE = build_nc()
    return _NC_CACHE


def prep_inputs(inputs):
    """Host-side shard + transpose + fp16 cast.  Returns per-core in_maps."""
    f32 = np.float32
    f16 = np.float16
    wq = np.asarray(inputs["wq"], f32)
    common = {
        "wqt16": np.ascontiguousarray((wq * 0.125).T).astype(f16),
        "wkt16": np.ascontiguousarray(np.asarray(inputs["wk"], f32).T).astype(f16),
        "wvt16": np.ascontiguousarray(np.asarray(inputs["wv"], f32).T).astype(f16),
        "fcwt16": np.ascontiguousarray(np.asarray(inputs["fc_w"], f32).T).astype(f16),
        "w1t16": np.ascontiguousarray(np.asarray(inputs["w1"], f32).T).astype(f16),
        "w2t16": np.ascontiguousarray(np.asarray(inputs["w2"], f32).T).astype(f16),
        "b1r": np.ascontiguousarray(
            np.asarray(inputs["b1"], f32).reshape(FT, 128).T),
        "lnrows16": np.stack([
            np.asarray(inputs["ln1_g"], f32),
            np.asarray(inputs["ln1_b"], f32),
            np.asarray(inputs["ln2_g"], f32),
            np.asarray(inputs["ln2_b"], f32),
            np.asarray(inputs["b2"], f32),
        ]).astype(f16),
    }
    enc = np.asarray(inputs["enc_input"], f32)
    cost = np.asarray(inputs["cost_mat"], f32)
    in_maps = []
    for b in range(NCORES):
        m = dict(common)
        m["x16"] = np.ascontiguousarray(enc[b]).astype(f16)
        m["xt16"] = np.ascontiguousarray(enc[b].T).astype(f16)
        m["costt16"] = np.ascontiguousarray(cost[b].T).astype(f16)
        in_maps.append(m)
    return in_maps


def kernel(**inputs):
    from concourse.bass_utils import run_bass_kernel_spmd

    nc = get_nc()
    in_maps = prep_inputs(inputs)
    res = run_bass_kernel_spmd(nc, in_maps, core_ids=list(range(NCORES)))
    return np.stack([res.results[b]["out"] for b in range(NCORES)], axis=0)
